# revision 14
# baseline (speedup 1.0000x reference)
"""Cost-volume kernel v3 for Trainium2 (8 NeuronCores, data-parallel over B*H).

cost[b,h,w,d] = mean_c left[b,h,w,c] * right[b,h,w-(d+1),c], 0 where w-d-1 < 0
Shapes: B=4, H=256, W=512, C=64, D=64 (f32). 1024 independent (b,h) rows,
128 per core.

v3 design (vs v2: M=64 quadrant matmuls, 127-col windows, 1.98x out pad):
  - Host packs inputs as bf16 in matmul-ready layout [128, pairs*W]:
    partition p = 64*(row parity) + c, free = pair*512 + w. Left pre-scaled
    by 1/C. Input 16.8 MB/core.
  - TensorE: M=32 col-tiled banded matmuls [K=64, M=32, N=95], 8-way PE
    tile packing via tile_position=(64*j2, 32*j). Window w' in [ws-64,
    ws+31); head windows (w'<0) shortened, clipped region left stale and
    masked to 0 on host. PE is LDWEIGHTS-area bound (~52-54 us/core, the
    same for any tiling that loads each left element once); it hides
    under DMA.
  - DVE/ACT alternate evicting psum [128, 380] -> bf16 rect in SBUF;
    rect DMAs contiguously to DRAM. Out rect ratio 95/64 = 1.48 ->
    12.45 MB/core (vs 16.65 in v2).
  - Host extracts the band with one as_strided view + cast per core:
      out[32j+q, (pair*2+j2)*380 + 95*wg + q + d']
        = cost[2*pair+j2, 128*wg+32j+q, 63-d']
Per-core traffic: in 16.8 + out 12.45 = 29.2 MB; mixed-direction DMA
sustains ~330-350 GB/s (HBM read+write cap; single direction ~417) ->
~85-88 us DMA floor, measured ~97 us steady per rep. DMA rings: inputs
on sync HWDGE, stores on scalar HWDGE (single-ring serialization and
SWDGE measured worse); in/store DMAs split in 2 per group to shorten
pipeline fill/drain.
"""

import numpy as np

N_CORES = 8
B_FULL, H_FULL, W, C = 4, 256, 512, 64
D = 64
ROWS = B_FULL * H_FULL           # 1024 independent rows
ROWS_PER_CORE = ROWS // N_CORES  # 128
PAIRS = ROWS_PER_CORE // 2       # 64 row pairs (2 rows share 128 partitions)
NBLK = W // 128                  # 4 w-blocks per row
BCOL = 2 * 127                   # rect cols per 128-w block (two rows)
FL = PAIRS * W                   # input free length per partition


def build_nc_v2(pg=4, lt_bufs=3, rect_bufs=3, ps_bufs=6, repeat=1,
                st_eng="scalar", rt_eng="sync", ev_vvs=False, store_split=1,
                skip_compute=False, skip_in=False, skip_store=False,
                mm_half=False, zp=False):
    import concourse.mybir as mybir
    import concourse.tile as tile
    from concourse import bacc

    nc = bacc.Bacc()
    left = nc.declare_dram_parameter("left", [128, FL], mybir.dt.bfloat16,
                                     isOutput=False)
    right = nc.declare_dram_parameter("right", [128, FL], mybir.dt.bfloat16,
                                      isOutput=False)
    ng = PAIRS // pg             # groups per core
    gcols = pg * NBLK * BCOL     # rect cols per group
    out = nc.declare_dram_parameter("out", [128, ng * gcols],
                                    mybir.dt.bfloat16, isOutput=True)

    with tile.TileContext(nc) as tc:
        with (
            tc.tile_pool(name="lt", bufs=lt_bufs) as lt_pool,
            tc.tile_pool(name="rt", bufs=lt_bufs) as rt_pool,
            tc.tile_pool(name="rect", bufs=rect_bufs) as rect_pool,
            tc.tile_pool(name="ps", bufs=ps_bufs, space="PSUM") as ps_pool,
        ):
          WR = W + 64 if zp else W     # Rt pair pitch (zp: 64 zero cols first)
          for _rep in range(repeat):
            for g in range(ng):
                f0 = g * pg * W
                Lt = lt_pool.tile([128, pg * W], mybir.dt.bfloat16, tag="lt")
                Rt = rt_pool.tile([128, pg * WR], mybir.dt.bfloat16, tag="rt")
                if not skip_in:
                    nc.sync.dma_start(Lt[:, :], left[:, f0:f0 + pg * W])
                    if zp:
                        # 64 zero head cols per pair segment (gpsimd, idle
                        # otherwise); DMA fills only the data region
                        for pr in range(pg):
                            nc.gpsimd.memset(
                                Rt[:, pr * WR:pr * WR + 64], 0.0)
                        for pr in range(pg):
                            getattr(nc, rt_eng).dma_start(
                                Rt[:, pr * WR + 64:(pr + 1) * WR],
                                right[:, f0 + pr * W:f0 + (pr + 1) * W])
                    else:
                        getattr(nc, rt_eng).dma_start(Rt[:, :],
                                                      right[:, f0:f0 + pg * W])

                Brect = rect_pool.tile([128, gcols], mybir.dt.bfloat16,
                                       tag="rect")
                if skip_in:
                    # keep tiles "written" so Tile release checks pass
                    nc.gpsimd.memset(Lt[:, 0:8], 0.0)
                    nc.gpsimd.memset(Rt[:, 0:8], 0.0)
                if skip_compute:
                    nc.gpsimd.memset(Brect[:, 0:8], 0.0)
                nev = 0
                for pr in range(pg if not skip_compute else 0):
                    rb = pr * W
                    rbR = pr * WR + 64 if zp else rb
                    # PE row-tiles (j=0/j=1 quadrant rows) must NOT write
                    # the same PSUM bank concurrently -> one bank per j,
                    # each collecting all 4 blocks of the pair.
                    PA = ps_pool.tile([128, 512], mybir.dt.float32, tag="ps")
                    PB = ps_pool.tile([128, 512], mybir.dt.float32, tag="ps")
                    PJ = [PA, PB]
                    for j in range(2):       # row of the pair (PE row tile)
                        P = PJ[j]
                        p0 = 64 * j
                        for i in range(NBLK):
                            cc = i * 127
                            for s in range(2):   # w half (PE col tile)
                                ws = i * 128 + 64 * s
                                pp = 64 * s
                                lhsT = Lt[p0:p0 + 64, rb + ws:rb + ws + 64]
                                if zp or not (i == 0 and s == 0):
                                    nw = 63 if mm_half else 127
                                    nc.tensor.matmul(
                                        P[pp:pp + 64, cc:cc + nw],
                                        lhsT,
                                        Rt[p0:p0 + 64,
                                           rbR + ws - 64:rbR + ws - 64 + nw],
                                        start=True, stop=True,
                                        tile_position=(p0, pp))
                                else:
                                    # w' < 0 head: zero, compute tail
                                    # (gpsimd/ACT cannot memset PSUM)
                                    nc.vector.memset(
                                        P[pp:pp + 64, cc:cc + 64], 0.0)
                                    nc.tensor.matmul(
                                        P[pp:pp + 64, cc + 64:cc + 127],
                                        lhsT,
                                        Rt[p0:p0 + 64, rb:rb + 63],
                                        start=True, stop=True,
                                        tile_position=(p0, pp))
                        col0 = (pr * 2 + j) * NBLK * 127
                        ev_dst = Brect[:, col0:col0 + NBLK * 127]
                        use_vec = (nev % 3 != 2) if ev_vvs else (nev % 2 == 0)
                        if use_vec:
                            nc.vector.tensor_copy(ev_dst,
                                                  P[:, 0:NBLK * 127])
                        else:
                            nc.scalar.copy(ev_dst, P[:, 0:NBLK * 127])
                        nev += 1

                if not skip_store:
                    cs = gcols // store_split
                    for sc in range(store_split):
                        getattr(nc, st_eng).dma_start(
                            out[:, g * gcols + sc * cs:
                                g * gcols + (sc + 1) * cs],
                            Brect[:, sc * cs:(sc + 1) * cs])

    nc.compile()
    return nc


def build_nc_v3(pg=4, lt_bufs=3, rect_bufs=3, ps_bufs=6, repeat=1,
                st_eng="scalar", rt_eng="sync", lt_eng="sync",
                store_split=1, st_lag=0, in_split=1, head_memset=False,
                skip_compute=False, skip_in=False, skip_store=False):
    """v3: M=32 col-tiled banded matmuls.

    Per (row, w-group of 128): 4 windows of 32 w each; window j covers
    w' in [ws-64, ws+30] (95 cols) so every d in [0,64) is present:
    psum[32j+q, 95g+n] = cost(w=128g+32j+q, w'=128g+32j-64+n), d=q+63-n.
    Stored rect ratio 95/64 = 1.48 (vs 127/64 = 1.98 in v2) -> out
    12.45 MB/core instead of 16.65. 8-way PE tile packing:
    tile_position=(64*j2, 32*j). Head windows (g=0, j<2) are shortened;
    the clipped region is stale psum, host masks w<=d to 0.
    """
    import concourse.mybir as mybir
    import concourse.tile as tile
    from concourse import bacc

    nc = bacc.Bacc()
    left = nc.declare_dram_parameter("left", [128, FL], mybir.dt.bfloat16,
                                     isOutput=False)
    right = nc.declare_dram_parameter("right", [128, FL], mybir.dt.bfloat16,
                                      isOutput=False)
    ng = PAIRS // pg              # pipeline groups per core
    PC = 4 * 95                   # psum/rect cols per row (4 w-groups)
    gcols = pg * 2 * PC           # rect cols per pipeline group
    out = nc.declare_dram_parameter("out", [128, ng * gcols],
                                    mybir.dt.bfloat16, isOutput=True)

    with tile.TileContext(nc) as tc:
        with (
            tc.tile_pool(name="lt", bufs=lt_bufs) as lt_pool,
            tc.tile_pool(name="rt", bufs=lt_bufs) as rt_pool,
            tc.tile_pool(name="rect", bufs=rect_bufs) as rect_pool,
            tc.tile_pool(name="ps", bufs=ps_bufs, space="PSUM") as ps_pool,
        ):
          for _rep in range(repeat):
            rects = []
            for g in range(ng):
                f0 = g * pg * W
                Lt = lt_pool.tile([128, pg * W], mybir.dt.bfloat16, tag="lt")
                Rt = rt_pool.tile([128, pg * W], mybir.dt.bfloat16, tag="rt")
                if not skip_in:
                    ic = pg * W // in_split
                    for sc in range(in_split):
                        getattr(nc, lt_eng).dma_start(
                            Lt[:, sc * ic:(sc + 1) * ic],
                            left[:, f0 + sc * ic:f0 + (sc + 1) * ic])
                        getattr(nc, rt_eng).dma_start(
                            Rt[:, sc * ic:(sc + 1) * ic],
                            right[:, f0 + sc * ic:f0 + (sc + 1) * ic])
                Brect = rect_pool.tile([128, gcols], mybir.dt.bfloat16,
                                       tag="rect")
                if skip_in:
                    nc.gpsimd.memset(Lt[:, 0:8], 0.0)
                    nc.gpsimd.memset(Rt[:, 0:8], 0.0)
                if skip_compute:
                    nc.gpsimd.memset(Brect[:, 0:8], 0.0)
                nev = 0
                for pr in range(pg if not skip_compute else 0):
                    rb = pr * W
                    PA = ps_pool.tile([128, PC], mybir.dt.float32, tag="ps")
                    PB = ps_pool.tile([128, PC], mybir.dt.float32, tag="ps")
                    PJ = [PA, PB]
                    if head_memset:
                        for j2 in range(2):
                            nc.vector.memset(PJ[j2][0:64, 0:64], 0.0)
                    for wg in range(4):       # w-group of 128 within the row
                        for j2 in range(2):   # row of the pair (PE row tile)
                            P = PJ[j2]
                            p0 = 64 * j2
                            for j in range(4):   # col tile (32 w each)
                                ws = 128 * wg + 32 * j
                                n0 = 64 - ws if ws < 64 else 0
                                nc.tensor.matmul(
                                    P[32 * j:32 * j + 32,
                                      95 * wg + n0:95 * wg + 95],
                                    Lt[p0:p0 + 64, rb + ws:rb + ws + 32],
                                    Rt[p0:p0 + 64,
                                       rb + ws - 64 + n0:rb + ws + 31],
                                    start=True, stop=True,
                                    tile_position=(p0, 32 * j))
                    for j2 in range(2):
                        col0 = (pr * 2 + j2) * PC
                        ev_dst = Brect[:, col0:col0 + PC]
                        if nev % 2 == 0:
                            nc.vector.tensor_copy(ev_dst, PJ[j2][:, 0:PC])
                        else:
                            nc.scalar.copy(ev_dst, PJ[j2][:, 0:PC])
                        nev += 1

                rects.append(Brect)
                if not skip_store:
                    if st_lag == 0:
                        _store_v3(nc, st_eng, out, Brect, g, gcols,
                                  store_split)
                    elif g >= st_lag:
                        # stagger: store of group g-st_lag rides the SAME
                        # ring as the input DMAs but st_lag groups behind,
                        # so the ring switches direction once per group
                        # instead of per packet.
                        _store_v3(nc, st_eng, out, rects[g - st_lag],
                                  g - st_lag, gcols, store_split)
            if not skip_store and st_lag > 0:
                for gt in range(ng - st_lag, ng):
                    _store_v3(nc, st_eng, out, rects[gt], gt, gcols,
                              store_split)

    nc.compile()
    return nc


def _store_v3(nc, st_eng, out, Brect, g, gcols, store_split):
    cs = gcols // store_split
    for sc in range(store_split):
        getattr(nc, st_eng).dma_start(
            out[:, g * gcols + sc * cs:g * gcols + (sc + 1) * cs],
            Brect[:, sc * cs:(sc + 1) * cs])


def _pack(x, scale):
    """[128 rows, W, C] f32 -> [128, PAIRS*W] bf16, p = 64*parity + c."""
    import ml_dtypes
    x = np.asarray(x, dtype=np.float32)
    if scale != 1.0:
        x = x * scale
    x = x.reshape(PAIRS, 2, W, C).transpose(1, 3, 0, 2)   # [j, c, pair, w]
    return np.ascontiguousarray(x.reshape(128, FL)).astype(ml_dtypes.bfloat16)


_NC_CACHE = {}


def _unshear(o, pg=4):
    """Device rect [128, ng*gcols] bf16 -> cost [128 rows, W, D] view-copy."""
    gcols = pg * NBLK * BCOL
    o = np.asarray(o).reshape(128, (PAIRS // pg) * gcols)
    es = o.strides[-1]           # element stride in bytes (2)
    # V[s, q, pr, j, i, d'] = o[64s+q, ((pr*2+j)*NBLK+i)*127 + q + d']
    V = np.lib.stride_tricks.as_strided(
        o,
        shape=(2, 64, PAIRS, 2, NBLK, 64),
        strides=(64 * o.shape[1] * es, (o.shape[1] + 1) * es,
                 2 * NBLK * 127 * es, NBLK * 127 * es, 127 * es, es))
    # -> [pr, j, i, s, q, d'] = [row, w, d'] ; flip d' -> d
    return V.transpose(2, 3, 4, 0, 1, 5).reshape(ROWS_PER_CORE, W, D)[:, :, ::-1]


def _unshear_v3(o, pg=4):
    """Device rect [128, ng*gcols] bf16 -> cost [128 rows, W, D] f32.

    o[32j+q, ((G*pg + pr)*2 + j2)*380 + 95*wg + q + d'] =
        cost[row=8G+2pr+j2, w=128*wg+32j+q, d=63-d'] ; w<=d region is
    stale garbage on device, masked to 0 here (exact per reference).
    """
    ng = PAIRS // pg
    PC = 4 * 95
    o = np.asarray(o).reshape(128, ng * pg * 2 * PC)
    RP = o.shape[1]
    es = o.strides[-1]
    V = np.lib.stride_tricks.as_strided(
        o,
        shape=(ng, pg, 2, 4, 4, 32, 64),
        strides=(pg * 2 * PC * es, 2 * PC * es, PC * es, 95 * es,
                 32 * RP * es, RP * es + es, es))
    # dims: (G, pr, j2, wg, j, q, d') -> rows=(G,pr,j2), w=(wg,j,q), d'
    cost = V.reshape(ROWS_PER_CORE, W, D)[:, :, ::-1].astype(np.float32)
    # clipped region (w - d - 1 < 0, only w < 64) is stale device garbage
    wd = np.arange(64)
    keep = wd[:, None] > wd[None, :]          # w > d
    cost[:, :64, :] = np.where(keep, cost[:, :64, :], np.float32(0.0))
    return cost


def kernel(left_feature, right_feature):
    from concourse.bass_utils import run_bass_kernel_spmd

    lf = np.asarray(left_feature, np.float32).reshape(ROWS, W, C)
    rf = np.asarray(right_feature, np.float32).reshape(ROWS, W, C)

    if "nc" not in _NC_CACHE:
        _NC_CACHE["nc"] = build_nc_v3(ps_bufs=8, lt_bufs=4, rect_bufs=4,
                                      in_split=2, store_split=2)
    nc = _NC_CACHE["nc"]

    in_maps = []
    for k in range(N_CORES):
        sl = slice(k * ROWS_PER_CORE, (k + 1) * ROWS_PER_CORE)
        in_maps.append({
            "left": _pack(lf[sl], 1.0 / C),
            "right": _pack(rf[sl], 1.0),
        })

    res = run_bass_kernel_spmd(nc, in_maps, core_ids=list(range(N_CORES)))

    out = np.empty((ROWS, W, D), dtype=np.float32)
    for k in range(N_CORES):
        out[k * ROWS_PER_CORE:(k + 1) * ROWS_PER_CORE] = _unshear_v3(
            res.results[k]["out"])
    return out.reshape(B_FULL, H_FULL, W, D)

